# revision 8
# baseline (speedup 1.0000x reference)
"""Multi-resolution 3D conv (3x3x3, Cin=Cout=16) on 8 TRN2 NeuronCores.

v3: int8 wire + natural layout + level-split (vs v2's z-split row layout).

- The axon tunnel (shared ~45-50MB/s, half-duplex-ish) is the bottleneck
  and the host has a SINGLE CPU, so the design minimizes both wire bytes
  and host-side work:
  * 8 cores = 4 batches x {64^3 level | 16^3+32^3+48^3 levels}. Each
    core's input is a CONTIGUOUS slab of the flat input in natural
    [z, y, x, C] layout, quantized to int8 on host (one vectorized pass)
    and uploaded as-is: zero halo, zero padding, zero host transposes.
  * The device gathers x-block tiles [128 parts = 8 x-positions x 16 Cin,
    z*y] straight from the natural layout via partition-innermost DMA
    access patterns (~15ms/core of device DMA time, hidden under the
    wire), converts i8->f16 exactly, and runs the same block-Toeplitz
    tap matmuls as v1/v2. SAME-conv z edges are handled by memset f16
    z-planes; x edges by partition memsets.
  * PSUM f32 = y/s_out - bias/s_out (scales folded into the fp16
    weights); one tensor_scalar_add with per-partition bias/s_out
    converts to int8 (RNE + saturation, HW-verified) and the output is
    scattered back to natural [z, y, x, C] int8, so host unpack is one
    vectorized dequant multiply into the f32 result.
  * Each core does ONE upload (fused [WT f16 | bias/s_out f32 | levels
    i8] buffer via DRAM bitcast) and ONE download.
- Wire: 28.0MB up + 26.2MB down = 54.2MB (v1: 83.4MB, v2: 56.2MB).
"""

import os
import sys
import threading
from concurrent.futures import ThreadPoolExecutor

for _p in ("/opt/trn_rl_repo",):
    if os.path.isdir(_p) and _p not in sys.path:
        sys.path.insert(0, _p)

import numpy as np

import concourse.bacc as bacc
import concourse.mybir as mybir
from concourse.tile import TileContext

RESOLUTIONS = (16, 32, 48, 64)
B, C = 4, 16
N_TOTAL = sum(r**3 for r in RESOLUTIONS)
XBO, XBI = 6, 8  # output / input x-positions per x-block
F16 = mybir.dt.float16
F32 = mybir.dt.float32
I8 = mybir.dt.int8

# nz: output z-rows per matmul chunk (nz * R <= 512, one PSUM bank)
NZ = {64: 8, 48: 8, 32: 16, 16: 8}

WT_C = 27 * 16 * 16  # compact weights: [t*3+d, ci, co] f16
HDR = WT_C * 2 + 96 * 4  # fused-buffer header bytes: compact WT f16 + scb f32

# quantization clip factors (in units of the respective sigma).
# Wider steps would lower the int8 stream's entropy (the axon tunnel
# entropy-codes uploads) but measured end-to-end gain was <1% while
# costing a third of the 2e-2 error headroom, so these stay at the
# clipping-optimal values (rel err 1.42e-2).
CLIP_IN = 4.0
CLIP_OUT = 4.3

# tap order: first tap is the full-width (b=1) one so its start=True
# write covers every PSUM element of the chunk
TAPS = [(0, 1), (0, 0), (0, 2), (1, 1), (1, 0), (1, 2), (2, 1), (2, 0), (2, 2)]

# level-split: program key -> levels (big level first keeps PE warm)
PROGS = {"A": (64,), "B": (48, 32, 16)}
# flat row offset of each level in [B, N_TOTAL, C]
LOFF = {}
_o = 0
for _R in RESOLUTIONS:
    LOFF[_R] = _o
    _o += _R**3
# per-program contiguous slab (rows) in the flat input
SLAB = {"A": (LOFF[64], LOFF[64] + 64**3), "B": (0, LOFF[64])}


def _xo(R, xb):
    return min(xb * XBO, R - XBO)


def _build_nc(levels):
    n_rows = sum(R**3 for R in levels)
    XN = HDR + n_rows * C  # fused upload bytes
    YN = n_rows * C
    nc = bacc.Bacc(target_bir_lowering=False)
    xin = nc.dram_tensor("xin", [XN], I8, kind="ExternalInput")
    yout = nc.dram_tensor("yout", [YN], I8, kind="ExternalOutput")

    wt = xin[0 : WT_C * 2].bitcast(F16).rearrange(
        "(k p o) -> k p o", p=16, o=16
    )
    scb = xin[WT_C * 2 : HDR].bitcast(F32).rearrange("(p o) -> p o", o=1)

    # natural-layout per-level views [q = x*16+ci, (z y)]
    xs, ys = {}, {}
    off = HDR
    for R in sorted(levels):  # levels stored in flat-offset order (small first)
        n = R**3 * C
        xs[R] = xin[off : off + n].rearrange("(z y q) -> q (z y)", q=R * C, z=R)
        ys[R] = yout[off - HDR : off - HDR + n].rearrange(
            "(z y q) -> q (z y)", q=R * C, z=R
        )
        off += n

    with TileContext(nc) as tc:
        with (
            tc.tile_pool(name="wp", bufs=1) as wp,
            tc.tile_pool(name="qp", bufs=4) as qp,
            tc.tile_pool(name="ip", bufs=4) as ip,
            tc.tile_pool(name="op", bufs=6) as op,
            tc.tile_pool(name="pp", bufs=4, space="PSUM") as pp,
            tc.tile_pool(name="dp", bufs=1, space="PSUM") as dp,
        ):
            # build the block-Toeplitz stationary WT on device from the
            # compact [27, 16, 16] upload: zero a staging tile, DMA the
            # 27*6 nonzero diagonal blocks in, then one DVE copy so the
            # tap matmuls have a single producer.
            wtst = wp.tile([128, 9, 96], F16, name="wtst")
            nc.vector.memset(wtst[:, :, :], 0.0)
            for t in range(9):
                for d in range(3):
                    for xo_ in range(XBO):
                        nc.sync.dma_start(
                            wtst[
                                (xo_ + d) * 16 : (xo_ + d) * 16 + 16,
                                t,
                                xo_ * 16 : xo_ * 16 + 16,
                            ],
                            wt[t * 3 + d],
                        )
            wt_sb = wp.tile([128, 9, 96], F16, name="wt_sb")
            nc.vector.tensor_copy(wt_sb[:, :, :], wtst[:, :, :])
            scr = wp.tile([96, 1], F32, name="scr")
            nc.sync.dma_start(scr[:, :], scb)
            # The S3 LDWEIGHTS slot only fits one semaphore wait, so make
            # sure every real matmul needs at most one: absorb each DMA's
            # completion wait with a tiny throwaway PE op first.
            dps = dp.tile([1, 2], F32, name="dps")
            nc.tensor.matmul(
                dps[0:1, 0:1], wt_sb[:, 0, 0:1], wt_sb[:, 0, 0:1],
                start=True, stop=True,
            )

            for R in levels:
                nz = NZ[R]
                Xp = -(-R // XBO)
                w_last = R - (Xp - 1) * XBO
                for xb in range(Xp):
                    xo = _xo(R, xb)
                    first = xb == 0
                    last = xb == Xp - 1
                    # int8 natural gather for this block's 128 partitions
                    pt = qp.tile([128, R * R], I8, tag="pt", name=f"pt{R}_{xb}")
                    if first:
                        # x = -1 does not exist: zero converts to 0.
                        # memset a 32-aligned span; DMA overwrites 16:32.
                        nc.vector.memset(pt[0:32, :], 0)
                        nc.sync.dma_start(pt[16:128, :], xs[R][0:112, :])
                    elif last:
                        nc.vector.memset(pt[96:128, :], 0)
                        nc.sync.dma_start(pt[0:112, :], xs[R][(R - 7) * C :, :])
                    else:
                        nc.sync.dma_start(
                            pt[:, :], xs[R][(xo - 1) * C : (xo - 1) * C + 128, :]
                        )
                    # f16 tile with one zero z-plane on each side (SAME pad)
                    it = ip.tile([128, R + 2, R], F16, tag="it", name=f"it{R}_{xb}")
                    nc.vector.memset(it[:, 0, :], 0.0)
                    nc.vector.memset(it[:, R + 1, :], 0.0)
                    nc.vector.tensor_copy(
                        it[:, 1 : R + 1, :].rearrange("p z y -> p (z y)"),
                        pt[:, :],
                    )
                    nc.tensor.matmul(
                        dps[0:1, 0:1], it[:, 0, 0:1], it[:, 0, 0:1],
                        start=True, stop=True,
                    )
                    for zc in range(0, R, nz):
                        ps = pp.tile([96, nz, R], F32, tag="ps", name=f"ps{R}_{xb}_{zc}")
                        for ti, (a, b) in enumerate(TAPS):
                            # SAME-conv y boundary: tap b contributes to
                            # out y in [max(0,1-b), R-b+1) cap [0, R)
                            ylo, yhi = (1, R) if b == 0 else (0, R - 1) if b == 2 else (0, R)
                            nc.tensor.matmul(
                                ps[:, :, ylo:yhi],
                                wt_sb[:, a * 3 + b, :],
                                it[:, zc + a : zc + a + nz, ylo + b - 1 : yhi + b - 1],
                                start=(ti == 0),
                                stop=(ti == 8),
                            )
                        # psum = y/s_out - bias/s_out; add per-partition
                        # bias/s_out and convert to int8 (RNE + saturate)
                        ot = op.tile([96, nz * R], I8, tag="ot", name=f"ot{R}_{xb}_{zc}")
                        nc.vector.tensor_scalar_add(
                            ot[:, :],
                            ps[:, :, :].rearrange("p z y -> p (z y)"),
                            scr[:, :],
                        )
                        if last:
                            skip = XBO - w_last
                            nc.sync.dma_start(
                                ys[R][(xo + skip) * C :, zc * R : (zc + nz) * R],
                                ot[skip * C :, :],
                            )
                        else:
                            nc.sync.dma_start(
                                ys[R][xo * C : xo * C + 96, zc * R : (zc + nz) * R],
                                ot[:, :],
                            )
    nc.finalize()
    return nc, XN, YN


def _build_wt(weight, s_ratio):
    # compact scaled weights [t*3+d, ci, co] = w[co,ci,a,b,d].T * s_ratio
    w = np.asarray(weight, np.float64) * s_ratio
    WC = np.empty((9, 3, C, C), np.float16)
    for t in range(9):
        a, b = t // 3, t % 3
        for d in range(3):
            WC[t, d] = w[:, :, a, b, d].T.astype(np.float16)
    return WC.reshape(-1).view(np.int8)


_ST = None  # lazy global state
_ST_LOCK = threading.Lock()


class _State:
    def __init__(self):
        import jax
        from concourse import bass2jax as b2j

        self.jax = jax
        b2j.install_neuronx_cc_hook()
        self.jfn = {}
        self.XN = {}
        self.YN = {}
        for key, levels in PROGS.items():
            nc, XN, YN = _build_nc(levels)
            part_name = (
                nc.partition_id_tensor.name
                if nc.partition_id_tensor is not None
                else None
            )
            in_names = ("xin", "yout") + ((part_name,) if part_name else ())
            out_aval = jax.core.ShapedArray((YN,), np.int8)

            def _body(xin_arr, ydummy, _nc=nc, _in=in_names, _oa=out_aval, _pn=part_name):
                operands = [xin_arr, ydummy]
                if _pn is not None:
                    operands.append(b2j.partition_id_tensor())
                outs = b2j._bass_exec_p.bind(
                    *operands,
                    out_avals=(_oa,),
                    in_names=_in,
                    out_names=("yout",),
                    lowering_input_output_aliases=(),
                    sim_require_finite=False,
                    sim_require_nnan=False,
                    nc=_nc,
                )
                return outs[0]

            self.jfn[key] = jax.jit(_body, keep_unused=True)
            self.XN[key] = XN
            self.YN[key] = YN

        self.devs = jax.devices()[:8]
        # core -> (program, batch): cores 0-3 = A(b0..b3), 4-7 = B(b0..b3)
        self.cmap = [("A", bi) for bi in range(B)] + [("B", bi) for bi in range(B)]

        # persistent on-device stand-ins for the zero-init output operand
        self.dummies = []
        for core, d in enumerate(self.devs):
            key = self.cmap[core][0]
            yn = self.YN[key]
            mk = jax.jit(lambda a, _yn=yn: jax.numpy.broadcast_to(a, (_yn,)))
            anchor = jax.device_put(np.zeros((), np.int8), d)
            self.dummies.append(jax.block_until_ready(mk(anchor)))

        # host buffers: per-core fused int8 upload rows + f32 quant scratch
        # + the reused full-shape output (avoids 105MB of fresh page
        # faults per call on the single host CPU)
        self.out = [np.empty((B, N_TOTAL, C), np.float32) for _ in range(2)]
        self.out_idx = 0
        self.XG = [np.empty(self.XN[self.cmap[c][0]], np.int8) for c in range(8)]
        self.tmp = [
            np.empty((SLAB[self.cmap[c][0]][1] - SLAB[self.cmap[c][0]][0]) * C,
                     np.float32)
            for c in range(8)
        ]
        self.pool = ThreadPoolExecutor(8)


def _get_state():
    global _ST
    if _ST is None:
        with _ST_LOCK:
            if _ST is None:
                _ST = _State()
    return _ST


def _pack_core(st, core, inp, inv_s_in, hdr):
    key, bi = st.cmap[core]
    row = st.XG[core]
    row[:HDR] = hdr
    r0, r1 = SLAB[key]
    slab = inp[bi, r0:r1].reshape(-1)
    t = st.tmp[core]
    np.multiply(slab, inv_s_in, out=t)
    np.rint(t, out=t)
    np.clip(t, -127, 127, out=t)
    np.copyto(row[HDR:], t, casting="unsafe")


def _unpack_core(st, core, ya, out, s_out):
    key, bi = st.cmap[core]
    r0, r1 = SLAB[key]
    dst = out[bi, r0:r1].reshape(-1)
    np.multiply(ya, np.float32(s_out), out=dst, casting="unsafe")


def _run(inputs, trace=False):
    st = _get_state()
    jax = st.jax
    inp = np.asarray(inputs["input"], np.float32)
    weight = np.asarray(inputs["weight"], np.float32)
    bias = np.asarray(inputs["bias"], np.float32)

    # scales: input sigma from a strided subsample; output sigma exactly
    # from the weights (y = sum w*x + b with x ~ N(0, s_x))
    sub = inp.reshape(-1)[:: 1001]
    sigma_x = float(np.sqrt(np.mean(sub * sub)))
    s_in = CLIP_IN * sigma_x / 127.0
    sig_y = np.sqrt((weight.astype(np.float64) ** 2).sum(axis=(1, 2, 3, 4)))
    s_out = float((np.abs(bias) + CLIP_OUT * sig_y * sigma_x).max() / 127.0)

    hdr = np.empty(HDR, np.int8)
    hdr[: WT_C * 2] = _build_wt(weight, s_in / s_out)
    hdr[WT_C * 2 :] = (
        np.tile((bias / s_out).astype(np.float32), XBO).view(np.int8)
    )

    out = st.out[st.out_idx]
    st.out_idx ^= 1
    inv_s_in = np.float32(1.0 / s_in)

    import time as _time

    t_base = _time.perf_counter()
    tl = [] if trace else None

    def _xfer(core):
        t1 = _time.perf_counter()
        key, _bi = st.cmap[core]
        dev = st.devs[core]
        x_dev = jax.device_put(st.XG[core], dev)
        jax.block_until_ready(x_dev)
        t2 = _time.perf_counter()
        y_dev = st.jfn[key](x_dev, st.dummies[core])
        ya = np.asarray(y_dev)
        t3 = _time.perf_counter()
        _unpack_core(st, core, ya, out, s_out)
        t4 = _time.perf_counter()
        if tl is not None:
            tl.append((core, t1 - t_base, t2 - t1, t3 - t2, t4 - t3))

    # pack sequentially on this thread (single host CPU); overlap the
    # wire via pool workers. Order A0,B0,A1,B1,... so big uploads start
    # early and each batch completes as soon as possible.
    order = [0, 4, 1, 5, 2, 6, 3, 7]
    futs = []
    for core in order:
        _pack_core(st, core, inp, inv_s_in, hdr)
        futs.append(st.pool.submit(_xfer, core))
    for f in futs:
        f.result()
    if tl is not None:
        for core, toff, tu, tx, tun in sorted(tl):
            print(
                f"  core{core}: start+{toff*1e3:6.1f} up {tu*1e3:6.1f} "
                f"exec+down {tx*1e3:6.1f} unpack {tun*1e3:6.1f}"
            )
    return out, None


def kernel(**inputs):
    out, _ = _run(inputs)
    return out


# revision 9
# speedup vs baseline: 1.0006x; 1.0006x over previous
"""Multi-resolution 3D conv (3x3x3, Cin=Cout=16) on 8 TRN2 NeuronCores.

v3: int8 wire + natural layout + level-split (vs v2's z-split row layout).

- The axon tunnel (shared ~45-50MB/s, half-duplex-ish) is the bottleneck
  and the host has a SINGLE CPU, so the design minimizes both wire bytes
  and host-side work:
  * 8 cores = 4 batches x {64^3 level | 16^3+32^3+48^3 levels}. Each
    core's input is a CONTIGUOUS slab of the flat input in natural
    [z, y, x, C] layout, quantized to int8 on host (one vectorized pass)
    and uploaded as-is: zero halo, zero padding, zero host transposes.
  * The device gathers x-block tiles [128 parts = 8 x-positions x 16 Cin,
    z*y] straight from the natural layout via partition-innermost DMA
    access patterns (~15ms/core of device DMA time, hidden under the
    wire), converts i8->f16 exactly, and runs the same block-Toeplitz
    tap matmuls as v1/v2. SAME-conv z edges are handled by memset f16
    z-planes; x edges by partition memsets.
  * PSUM f32 = y/s_out - bias/s_out (scales folded into the fp16
    weights); one tensor_scalar_add with per-partition bias/s_out
    converts to int8 (RNE + saturation, HW-verified) and the output is
    scattered back to natural [z, y, x, C] int8, so host unpack is one
    vectorized dequant multiply into the f32 result.
  * Each core does ONE upload (fused [compact 27x16x16 weights f16 |
    bias/s_out f32 | levels i8] buffer via DRAM bitcast) and ONE
    download. The block-Toeplitz stationary WT is built on device from
    the 14KB compact weights (162 diagonal-block DMAs into a staging
    tile + one DVE copy as single producer).
- Wire: 26.3MB up + 26.2MB down = 52.5MB (v1: 83.4MB, v2: 56.2MB),
  running at the tunnel's measured ~50MB/s payload ceiling.
"""

import os
import sys
import threading
from concurrent.futures import ThreadPoolExecutor

for _p in ("/opt/trn_rl_repo",):
    if os.path.isdir(_p) and _p not in sys.path:
        sys.path.insert(0, _p)

import numpy as np

import concourse.bacc as bacc
import concourse.mybir as mybir
from concourse.tile import TileContext

RESOLUTIONS = (16, 32, 48, 64)
B, C = 4, 16
N_TOTAL = sum(r**3 for r in RESOLUTIONS)
XBO, XBI = 6, 8  # output / input x-positions per x-block
F16 = mybir.dt.float16
F32 = mybir.dt.float32
I8 = mybir.dt.int8

# nz: output z-rows per matmul chunk (nz * R <= 512, one PSUM bank)
NZ = {64: 8, 48: 8, 32: 16, 16: 8}

WT_C = 27 * 16 * 16  # compact weights: [t*3+d, ci, co] f16
HDR = WT_C * 2 + 96 * 4  # fused-buffer header bytes: compact WT f16 + scb f32

# quantization clip factors (in units of the respective sigma).
# Wider steps would lower the int8 stream's entropy (the axon tunnel
# entropy-codes uploads) but measured end-to-end gain was <1% while
# costing a third of the 2e-2 error headroom, so these stay at the
# clipping-optimal values (rel err 1.42e-2).
CLIP_IN = 4.0
CLIP_OUT = 4.3

# tap order: first tap is the full-width (b=1) one so its start=True
# write covers every PSUM element of the chunk
TAPS = [(0, 1), (0, 0), (0, 2), (1, 1), (1, 0), (1, 2), (2, 1), (2, 0), (2, 2)]

# level-split: program key -> levels (big level first keeps PE warm)
PROGS = {"A": (64,), "B": (48, 32, 16)}
# flat row offset of each level in [B, N_TOTAL, C]
LOFF = {}
_o = 0
for _R in RESOLUTIONS:
    LOFF[_R] = _o
    _o += _R**3
# per-program contiguous slab (rows) in the flat input
SLAB = {"A": (LOFF[64], LOFF[64] + 64**3), "B": (0, LOFF[64])}


def _xo(R, xb):
    return min(xb * XBO, R - XBO)


def _build_nc(levels):
    n_rows = sum(R**3 for R in levels)
    XN = HDR + n_rows * C  # fused upload bytes
    YN = n_rows * C
    nc = bacc.Bacc(target_bir_lowering=False)
    xin = nc.dram_tensor("xin", [XN], I8, kind="ExternalInput")
    yout = nc.dram_tensor("yout", [YN], I8, kind="ExternalOutput")

    wt = xin[0 : WT_C * 2].bitcast(F16).rearrange(
        "(k p o) -> k p o", p=16, o=16
    )
    scb = xin[WT_C * 2 : HDR].bitcast(F32).rearrange("(p o) -> p o", o=1)

    # natural-layout per-level views [q = x*16+ci, (z y)]
    xs, ys = {}, {}
    off = HDR
    for R in sorted(levels):  # levels stored in flat-offset order (small first)
        n = R**3 * C
        xs[R] = xin[off : off + n].rearrange("(z y q) -> q (z y)", q=R * C, z=R)
        ys[R] = yout[off - HDR : off - HDR + n].rearrange(
            "(z y q) -> q (z y)", q=R * C, z=R
        )
        off += n

    with TileContext(nc) as tc:
        with (
            tc.tile_pool(name="wp", bufs=1) as wp,
            tc.tile_pool(name="qp", bufs=4) as qp,
            tc.tile_pool(name="ip", bufs=4) as ip,
            tc.tile_pool(name="op", bufs=6) as op,
            tc.tile_pool(name="pp", bufs=4, space="PSUM") as pp,
            tc.tile_pool(name="dp", bufs=1, space="PSUM") as dp,
        ):
            # build the block-Toeplitz stationary WT on device from the
            # compact [27, 16, 16] upload: zero a staging tile, DMA the
            # 27*6 nonzero diagonal blocks in, then one DVE copy so the
            # tap matmuls have a single producer.
            wtst = wp.tile([128, 9, 96], F16, name="wtst")
            nc.vector.memset(wtst[:, :, :], 0.0)
            for t in range(9):
                for d in range(3):
                    for xo_ in range(XBO):
                        nc.sync.dma_start(
                            wtst[
                                (xo_ + d) * 16 : (xo_ + d) * 16 + 16,
                                t,
                                xo_ * 16 : xo_ * 16 + 16,
                            ],
                            wt[t * 3 + d],
                        )
            wt_sb = wp.tile([128, 9, 96], F16, name="wt_sb")
            nc.vector.tensor_copy(wt_sb[:, :, :], wtst[:, :, :])
            scr = wp.tile([96, 1], F32, name="scr")
            nc.sync.dma_start(scr[:, :], scb)
            # The S3 LDWEIGHTS slot only fits one semaphore wait, so make
            # sure every real matmul needs at most one: absorb each DMA's
            # completion wait with a tiny throwaway PE op first.
            dps = dp.tile([1, 2], F32, name="dps")
            nc.tensor.matmul(
                dps[0:1, 0:1], wt_sb[:, 0, 0:1], wt_sb[:, 0, 0:1],
                start=True, stop=True,
            )

            for R in levels:
                nz = NZ[R]
                Xp = -(-R // XBO)
                w_last = R - (Xp - 1) * XBO
                for xb in range(Xp):
                    xo = _xo(R, xb)
                    first = xb == 0
                    last = xb == Xp - 1
                    # int8 natural gather for this block's 128 partitions
                    pt = qp.tile([128, R * R], I8, tag="pt", name=f"pt{R}_{xb}")
                    if first:
                        # x = -1 does not exist: zero converts to 0.
                        # memset a 32-aligned span; DMA overwrites 16:32.
                        nc.vector.memset(pt[0:32, :], 0)
                        nc.sync.dma_start(pt[16:128, :], xs[R][0:112, :])
                    elif last:
                        nc.vector.memset(pt[96:128, :], 0)
                        nc.sync.dma_start(pt[0:112, :], xs[R][(R - 7) * C :, :])
                    else:
                        nc.sync.dma_start(
                            pt[:, :], xs[R][(xo - 1) * C : (xo - 1) * C + 128, :]
                        )
                    # f16 tile with one zero z-plane on each side (SAME pad)
                    it = ip.tile([128, R + 2, R], F16, tag="it", name=f"it{R}_{xb}")
                    nc.vector.memset(it[:, 0, :], 0.0)
                    nc.vector.memset(it[:, R + 1, :], 0.0)
                    nc.vector.tensor_copy(
                        it[:, 1 : R + 1, :].rearrange("p z y -> p (z y)"),
                        pt[:, :],
                    )
                    nc.tensor.matmul(
                        dps[0:1, 0:1], it[:, 0, 0:1], it[:, 0, 0:1],
                        start=True, stop=True,
                    )
                    for zc in range(0, R, nz):
                        ps = pp.tile([96, nz, R], F32, tag="ps", name=f"ps{R}_{xb}_{zc}")
                        for ti, (a, b) in enumerate(TAPS):
                            # SAME-conv y boundary: tap b contributes to
                            # out y in [max(0,1-b), R-b+1) cap [0, R)
                            ylo, yhi = (1, R) if b == 0 else (0, R - 1) if b == 2 else (0, R)
                            nc.tensor.matmul(
                                ps[:, :, ylo:yhi],
                                wt_sb[:, a * 3 + b, :],
                                it[:, zc + a : zc + a + nz, ylo + b - 1 : yhi + b - 1],
                                start=(ti == 0),
                                stop=(ti == 8),
                            )
                        # psum = y/s_out - bias/s_out; add per-partition
                        # bias/s_out and convert to int8 (RNE + saturate)
                        ot = op.tile([96, nz * R], I8, tag="ot", name=f"ot{R}_{xb}_{zc}")
                        nc.vector.tensor_scalar_add(
                            ot[:, :],
                            ps[:, :, :].rearrange("p z y -> p (z y)"),
                            scr[:, :],
                        )
                        if last:
                            skip = XBO - w_last
                            nc.sync.dma_start(
                                ys[R][(xo + skip) * C :, zc * R : (zc + nz) * R],
                                ot[skip * C :, :],
                            )
                        else:
                            nc.sync.dma_start(
                                ys[R][xo * C : xo * C + 96, zc * R : (zc + nz) * R],
                                ot[:, :],
                            )
    nc.finalize()
    return nc, XN, YN


def _build_wt(weight, s_ratio):
    # compact scaled weights [t*3+d, ci, co] = w[co,ci,a,b,d].T * s_ratio
    w = np.asarray(weight, np.float64) * s_ratio
    WC = np.empty((9, 3, C, C), np.float16)
    for t in range(9):
        a, b = t // 3, t % 3
        for d in range(3):
            WC[t, d] = w[:, :, a, b, d].T.astype(np.float16)
    return WC.reshape(-1).view(np.int8)


_ST = None  # lazy global state
_ST_LOCK = threading.Lock()


class _State:
    def __init__(self):
        import jax
        from concourse import bass2jax as b2j

        self.jax = jax
        b2j.install_neuronx_cc_hook()
        self.jfn = {}
        self.XN = {}
        self.YN = {}
        for key, levels in PROGS.items():
            nc, XN, YN = _build_nc(levels)
            part_name = (
                nc.partition_id_tensor.name
                if nc.partition_id_tensor is not None
                else None
            )
            in_names = ("xin", "yout") + ((part_name,) if part_name else ())
            out_aval = jax.core.ShapedArray((YN,), np.int8)

            def _body(xin_arr, ydummy, _nc=nc, _in=in_names, _oa=out_aval, _pn=part_name):
                operands = [xin_arr, ydummy]
                if _pn is not None:
                    operands.append(b2j.partition_id_tensor())
                outs = b2j._bass_exec_p.bind(
                    *operands,
                    out_avals=(_oa,),
                    in_names=_in,
                    out_names=("yout",),
                    lowering_input_output_aliases=(),
                    sim_require_finite=False,
                    sim_require_nnan=False,
                    nc=_nc,
                )
                return outs[0]

            self.jfn[key] = jax.jit(_body, keep_unused=True)
            self.XN[key] = XN
            self.YN[key] = YN

        self.devs = jax.devices()[:8]
        # core -> (program, batch): cores 0-3 = A(b0..b3), 4-7 = B(b0..b3)
        self.cmap = [("A", bi) for bi in range(B)] + [("B", bi) for bi in range(B)]

        # persistent on-device stand-ins for the zero-init output operand
        self.dummies = []
        for core, d in enumerate(self.devs):
            key = self.cmap[core][0]
            yn = self.YN[key]
            mk = jax.jit(lambda a, _yn=yn: jax.numpy.broadcast_to(a, (_yn,)))
            anchor = jax.device_put(np.zeros((), np.int8), d)
            self.dummies.append(jax.block_until_ready(mk(anchor)))

        # host buffers: per-core fused int8 upload rows + f32 quant scratch
        # + the reused full-shape output (avoids 105MB of fresh page
        # faults per call on the single host CPU)
        self.out = [np.empty((B, N_TOTAL, C), np.float32) for _ in range(2)]
        self.out_idx = 0
        self.XG = [np.empty(self.XN[self.cmap[c][0]], np.int8) for c in range(8)]
        self.tmp = [
            np.empty((SLAB[self.cmap[c][0]][1] - SLAB[self.cmap[c][0]][0]) * C,
                     np.float32)
            for c in range(8)
        ]
        self.pool = ThreadPoolExecutor(8)


def _get_state():
    global _ST
    if _ST is None:
        with _ST_LOCK:
            if _ST is None:
                _ST = _State()
    return _ST


def _pack_core(st, core, inp, inv_s_in, hdr):
    key, bi = st.cmap[core]
    row = st.XG[core]
    row[:HDR] = hdr
    r0, r1 = SLAB[key]
    slab = inp[bi, r0:r1].reshape(-1)
    t = st.tmp[core]
    np.multiply(slab, inv_s_in, out=t)
    np.rint(t, out=t)
    np.clip(t, -127, 127, out=t)
    np.copyto(row[HDR:], t, casting="unsafe")


def _unpack_core(st, core, ya, out, s_out):
    key, bi = st.cmap[core]
    r0, r1 = SLAB[key]
    dst = out[bi, r0:r1].reshape(-1)
    np.multiply(ya, np.float32(s_out), out=dst, casting="unsafe")


def _run(inputs, trace=False):
    st = _get_state()
    jax = st.jax
    inp = np.asarray(inputs["input"], np.float32)
    weight = np.asarray(inputs["weight"], np.float32)
    bias = np.asarray(inputs["bias"], np.float32)

    # scales: input sigma from a strided subsample; output sigma exactly
    # from the weights (y = sum w*x + b with x ~ N(0, s_x))
    sub = inp.reshape(-1)[:: 1001]
    sigma_x = float(np.sqrt(np.mean(sub * sub)))
    s_in = CLIP_IN * sigma_x / 127.0
    sig_y = np.sqrt((weight.astype(np.float64) ** 2).sum(axis=(1, 2, 3, 4)))
    s_out = float((np.abs(bias) + CLIP_OUT * sig_y * sigma_x).max() / 127.0)

    hdr = np.empty(HDR, np.int8)
    hdr[: WT_C * 2] = _build_wt(weight, s_in / s_out)
    hdr[WT_C * 2 :] = (
        np.tile((bias / s_out).astype(np.float32), XBO).view(np.int8)
    )

    out = st.out[st.out_idx]
    st.out_idx ^= 1
    inv_s_in = np.float32(1.0 / s_in)

    import time as _time

    t_base = _time.perf_counter()
    tl = [] if trace else None

    def _xfer(core):
        t1 = _time.perf_counter()
        key, _bi = st.cmap[core]
        dev = st.devs[core]
        x_dev = jax.device_put(st.XG[core], dev)
        jax.block_until_ready(x_dev)
        t2 = _time.perf_counter()
        y_dev = st.jfn[key](x_dev, st.dummies[core])
        ya = np.asarray(y_dev)
        t3 = _time.perf_counter()
        _unpack_core(st, core, ya, out, s_out)
        t4 = _time.perf_counter()
        if tl is not None:
            tl.append((core, t1 - t_base, t2 - t1, t3 - t2, t4 - t3))

    # pack sequentially on this thread (single host CPU); overlap the
    # wire via pool workers. Order A0,B0,A1,B1,... so big uploads start
    # early and each batch completes as soon as possible.
    order = [0, 4, 1, 5, 2, 6, 3, 7]
    futs = []
    for core in order:
        _pack_core(st, core, inp, inv_s_in, hdr)
        futs.append(st.pool.submit(_xfer, core))
    for f in futs:
        f.result()
    if tl is not None:
        for core, toff, tu, tx, tun in sorted(tl):
            print(
                f"  core{core}: start+{toff*1e3:6.1f} up {tu*1e3:6.1f} "
                f"exec+down {tx*1e3:6.1f} unpack {tun*1e3:6.1f}"
            )
    return out, None


def kernel(**inputs):
    out, _ = _run(inputs)
    return out
